# revision 21
# baseline (speedup 1.0000x reference)
"""Bahdanau attention Trainium2 Bass kernel (v2).

Problem (fixed shapes):
  decoder_state [32, 1024] f32, encoder_hiddens [32, 2048, 1024] f32,
  Wa_w [1,1024], Wa_b [1], Wb_w [1024,1024], Wb_b [1024], Wc_w [1024,1024], Wc_b [1024]
  out: context [32, 1024] f32

Strategy: data-parallel over batch, 4 batches per core on 8 cores.

Host-side prep is layout-only: cast encoder_hiddens to bf16 (keeps DMA at
half volume) and pre-transpose/cast the small weight matrices into the
[h-partition, tile, free] layouts the PE wants. On-chip there are NO
enc/weight transposes: encoder blocks are loaded with the DMA XBAR
transpose (16-bit path) directly into [h, s] tiles.

Per 512-wide s-block: enc_proj k-tiles via bf16 matmuls (8 ht-accumulated
groups, moving dim 512), tanh(+dec_proj bias) on the Act engine into bf16
e-tiles, the Wa score reduction as a DVE multiply-accumulate over
k-tiles followed by a single ones-vector matmul (cross-partition sum),
exp without max subtraction (scores are O(+-5) for this input
distribution, fp32 exp is exact there; softmax shift-invariance makes
max subtraction optional), and the context partial on the DVE from the
same encT tiles. Score/context stages for block i are deferred into
block i+1's matmul stream so the PE never waits on them.
"""
import sys

if "/opt/trn_rl_repo" not in sys.path:
    sys.path.insert(0, "/opt/trn_rl_repo")

import numpy as np
import ml_dtypes

import concourse.bass as bass
import concourse.tile as tile
from concourse import bacc, mybir
from concourse import bass_utils
from concourse.masks import make_identity

F32 = mybir.dt.float32
F32R = mybir.dt.float32r
BF16 = mybir.dt.bfloat16
F16 = mybir.dt.float16

B, S, H, K = 32, 2048, 1024, 1024
NCORES = 8
BLOC = B // NCORES          # batches per core
SBLK = 512                  # s-block width
NBLK = S // SBLK            # 4
NHT = H // 128              # 8
NKT = K // 128              # 8
NIT = BLOC * NBLK           # 16 block iterations

# chunk schedule: (batch, s-offset, width, chunk-index-in-batch). The last
# block of the last batch is split into two 256-wide chunks to halve the
# serial score->context->combine chain at the kernel tail.
SCHED = []
for _b in range(BLOC):
    if _b < BLOC - 1:
        SCHED += [(_b, _blk * SBLK, SBLK, _blk) for _blk in range(NBLK)]
    else:
        SCHED += [(_b, 0, SBLK, 0), (_b, SBLK, SBLK, 1),
                  (_b, 2 * SBLK, SBLK, 2),
                  (_b, 3 * SBLK, 256, 3), (_b, 3 * SBLK + 256, 256, 4)]


def build_kernel():
    nc = bacc.Bacc("TRN2", target_bir_lowering=False)

    enc = nc.dram_tensor("enc", [BLOC, S, H], BF16, kind="ExternalInput")
    wct = nc.dram_tensor("wct", [128, NHT, K], BF16, kind="ExternalInput")
    wbt = nc.dram_tensor("wbt", [128, NHT, K], BF16, kind="ExternalInput")
    dect = nc.dram_tensor("dect", [128, NHT, BLOC], BF16, kind="ExternalInput")
    wat = nc.dram_tensor("wat", [128, NKT], F32, kind="ExternalInput")
    bias = nc.dram_tensor("bias", [128, NKT], F32, kind="ExternalInput")
    y = nc.dram_tensor("y", [BLOC, H], F32, kind="ExternalOutput")

    TT = mybir.ActivationFunctionType.Tanh
    EX = mybir.ActivationFunctionType.Exp
    ADD = mybir.AluOpType.add
    MULT = mybir.AluOpType.mult

    from contextlib import ExitStack
    with tile.TileContext(nc) as tc, ExitStack() as stack:
        consts = stack.enter_context(tc.tile_pool(name="consts", bufs=1))
        identf = consts.tile([128, 128], F32)
        make_identity(nc, identf)
        ones_col = consts.tile([128, 1], F16)
        nc.vector.memset(ones_col, 1.0)
        wcT = consts.tile([128, NHT, K], BF16)
        waT = consts.tile([128, NKT], F32)
        bseg = consts.tile([128, NKT], F32)
        decT = consts.tile([128, NHT, BLOC], BF16)
        bias_kb = consts.tile([128, NKT, BLOC], F32)

        enc_p = stack.enter_context(tc.tile_pool(name="encT", bufs=5))

        def load_enc(i):
            b, off, w, ci = SCHED[i]
            t = enc_p.tile([128, NHT, SBLK], BF16, tag="eT")
            nc.sync.dma_start_transpose(
                t[:, :, 0:w], enc[b, off:off + w, :])
            return t

        # ---------------- setup: dec_proj -> bias_kb ----------------
        # DMA order: wbT leads the sync ring (it gates the dec_proj -> bias
        # chain, which must beat the first tanh); the enc-block XBAR stream
        # follows it. wcT + smalls go on the scalar ring in parallel.
        setup_ctx = ExitStack()
        setup = setup_ctx.enter_context(tc.tile_pool(name="setup", bufs=1))
        sps = setup_ctx.enter_context(
            tc.tile_pool(name="setup_ps", bufs=1, space="PSUM"))
        if True:
            wbT = setup.tile([128, NHT, K], BF16, tag="wbT")
            nc.sync.dma_start(out=wbT[:, :, 0:512], in_=wbt[:, :, 0:512])
            nc.sync.dma_start(out=wbT[:, :, 512:K], in_=wbt[:, :, 512:K])
            nc.scalar.dma_start(out=decT, in_=dect[:, :, :])
            nc.scalar.dma_start(out=waT, in_=wat[:, :])
            nc.scalar.dma_start(out=bseg, in_=bias[:, :])
            nc.scalar.dma_start(out=wcT, in_=wct[:, :, :])
            encT = {0: load_enc(0), 1: load_enc(1)}

            dp_row = setup.tile([BLOC, K], F32, tag="dp_row")
            for half in range(2):
                psd = sps.tile([BLOC, 512], F32, tag=f"psd{half}")
                for ht in range(NHT):
                    nc.tensor.matmul(psd, decT[:, ht, :],
                                     wbT[:, ht, half * 512:(half + 1) * 512],
                                     start=(ht == 0), stop=(ht == NHT - 1))
                nc.vector.tensor_copy(dp_row[:, half * 512:(half + 1) * 512], psd)
            pst = sps.tile([128, NKT, BLOC], F32, tag="pst")
            for kt in range(NKT):
                nc.tensor.transpose(pst[:, kt, :],
                                    dp_row[:, kt * 128:(kt + 1) * 128],
                                    identf[0:BLOC, 0:BLOC])
            for kt in range(NKT):
                bs = bseg[:, kt:kt + 1]
                nc.vector.tensor_tensor(
                    out=bias_kb[:, kt, :], in0=pst[:, kt, :],
                    in1=bass.AP(tensor=bs.tensor, offset=bs.offset,
                                ap=[bs.ap[0], [0, BLOC]]),
                    op=ADD)
        setup_ctx.close()

        # ---------------- main loop pools ----------------
        e_p = stack.enter_context(tc.tile_pool(name="e", bufs=10))
        acc_p = stack.enter_context(tc.tile_pool(name="acc", bufs=2))
        wbt_p = stack.enter_context(tc.tile_pool(name="wbcast", bufs=2))
        scr_p = stack.enter_context(tc.tile_pool(name="scr", bufs=2))
        row_p = stack.enter_context(tc.tile_pool(name="rows", bufs=3))
        stat_p = stack.enter_context(tc.tile_pool(name="stats", bufs=6))
        ctx_p = stack.enter_context(tc.tile_pool(name="ctxT", bufs=8))
        ysb_p = stack.enter_context(tc.tile_pool(name="ysb", bufs=2))
        ps_e = stack.enter_context(tc.tile_pool(name="ps_e", bufs=6, space="PSUM"))
        ps_s = stack.enter_context(tc.tile_pool(name="ps_s", bufs=1, space="PSUM"))
        ps_y = stack.enter_context(tc.tile_pool(name="ps_y", bufs=1, space="PSUM"))

        def flush_scores(task):
            # chunk i's scores: cross-partition sum of acc via ones-matmul,
            # then exp (no max subtraction; see module docstring).
            i, acc, zrow, ctx_blks, w, ci = task
            pss = ps_s.tile([1, SBLK], F32, tag="pss")
            nc.tensor.matmul(pss[:, 0:w], ones_col, acc[:, 0:w],
                             start=True, stop=True)
            wrow = row_p.tile([1, SBLK], BF16, tag="wrow")
            nc.scalar.activation(wrow[:, 0:w], pss[:, 0:w], EX,
                                 accum_out=zrow[:, ci:ci + 1])
            return wrow

        def flush_context(task, wrow):
            # chunk i's context partial on DVE from the encT tiles.
            i, acc, zrow, ctx_blks, w, ci = task
            wb_t = wbt_p.tile([128, SBLK], BF16, tag="wb")
            nc.gpsimd.partition_broadcast(wb_t[:, 0:w], wrow[:, 0:w], 128)
            ctxT = ctx_p.tile([128, NHT], F32, tag="ct")
            for ht in range(NHT):
                scr = scr_p.tile([128, SBLK], BF16, tag="scr")
                nc.vector.scalar_tensor_tensor(
                    out=scr[:, 0:w], in0=encT[i][:, ht, 0:w], scalar=1.0,
                    in1=wb_t[:, 0:w],
                    op0=MULT, op1=MULT, accum_out=ctxT[:, ht:ht + 1])
            ctx_blks.append(ctxT)

        def flush_combine(task):
            b, zrow, ctx_blks = task
            z = stat_p.tile([1, 1], F32, tag="z")
            nc.vector.reduce_sum(z, zrow[:, 0:len(ctx_blks)],
                                 axis=mybir.AxisListType.X)
            rz = stat_p.tile([1, 1], F32, tag="rz")
            nc.vector.reciprocal(rz, z)
            rzB = stat_p.tile([128, 1], F32, tag="rzB")
            nc.gpsimd.partition_broadcast(rzB, rz, 128)
            s01 = stat_p.tile([128, NHT], F32, tag="s01")
            nc.vector.tensor_tensor(out=s01, in0=ctx_blks[0], in1=ctx_blks[1], op=ADD)
            s23 = stat_p.tile([128, NHT], F32, tag="s23")
            nc.vector.tensor_tensor(out=s23, in0=ctx_blks[2], in1=ctx_blks[3], op=ADD)
            stot = stat_p.tile([128, NHT], F32, tag="stot")
            nc.vector.tensor_tensor(out=stot, in0=s01, in1=s23, op=ADD)
            if len(ctx_blks) > 4:
                stot5 = stat_p.tile([128, NHT], F32, tag="stot5")
                nc.vector.tensor_tensor(out=stot5, in0=stot, in1=ctx_blks[4], op=ADD)
                stot = stot5
            ys = stat_p.tile([128, NHT], F32, tag="ys")
            nc.vector.tensor_scalar_mul(ys, stot, rzB)
            psy = ps_y.tile([NHT, 128], F32, tag="psy")
            nc.tensor.transpose(psy, ys, identf)
            yrow = ysb_p.tile([NHT, 128], F32, tag="yrow")
            nc.vector.tensor_copy(yrow, psy)
            nc.scalar.dma_start(
                out=y[b:b + 1, :].rearrange("o (ht hp) -> (o ht) hp", hp=128),
                in_=yrow)

        pending = None        # task awaiting scores+context
        pending_comb = None   # (b, zrow, ctx_blks) awaiting final combine
        zrow = None
        ctx_blks = None
        for i, (b, off, w, ci) in enumerate(SCHED):
            if ci == 0:
                zrow = stat_p.tile([1, NBLK + 1], F32, tag="zrow")
                ctx_blks = []
            if i + 2 < len(SCHED):
                encT[i + 2] = load_enc(i + 2)

            acc = acc_p.tile([128, SBLK], F16, tag="acc")
            for kt in range(NKT):
                pse = ps_e.tile([128, SBLK], F32, tag="pe")
                for ht in range(NHT):
                    nc.tensor.matmul(pse[:, 0:w],
                                     wcT[:, ht, kt * 128:(kt + 1) * 128],
                                     encT[i][:, ht, 0:w],
                                     start=(ht == 0), stop=(ht == NHT - 1))
                et = e_p.tile([128, SBLK], BF16, tag="et")
                nc.scalar.activation(et[:, 0:w], pse[:, 0:w], TT,
                                     bias=bias_kb[:, kt, b:b + 1])
                if kt == 0:
                    nc.vector.tensor_scalar_mul(acc[:, 0:w], et[:, 0:w],
                                                waT[:, 0:1])
                else:
                    nc.vector.scalar_tensor_tensor(
                        out=acc[:, 0:w], in0=et[:, 0:w],
                        scalar=waT[:, kt:kt + 1], in1=acc[:, 0:w],
                        op0=MULT, op1=ADD)

                if kt == 2 and pending is not None:
                    wrow = flush_scores(pending)
                if kt == 7 and pending is not None:
                    # after this block's last acc op: the score-critical DVE
                    # chain stays ahead of the context/combine backlog.
                    flush_context(pending, wrow)
                    del encT[pending[0]]
                    pending = None
                    if pending_comb is not None:
                        flush_combine(pending_comb)
                        pending_comb = None

            pending = (i, acc, zrow, ctx_blks, w, ci)
            if i + 1 == len(SCHED) or SCHED[i + 1][3] == 0:
                pending_comb = (b, zrow, ctx_blks)

        wrow = flush_scores(pending)
        flush_context(pending, wrow)
        flush_combine(pending_comb)

    nc.compile()
    return nc


_NC_CACHE = None


def _get_nc():
    global _NC_CACHE
    if _NC_CACHE is None:
        _NC_CACHE = build_kernel()
    return _NC_CACHE


def _prep_weights(Wa_w, Wb_w, Wb_b, Wc_w, Wc_b):
    # [h, k] transposed weights, rows regrouped to [128, NHT, K] with
    # h = ht*128 + p (matches the XBAR DMA-transpose layout of enc tiles).
    wcT = np.ascontiguousarray(Wc_w.T).astype(ml_dtypes.bfloat16)
    wbT = np.ascontiguousarray(Wb_w.T).astype(ml_dtypes.bfloat16)
    wct = np.ascontiguousarray(wcT.reshape(NHT, 128, K).transpose(1, 0, 2))
    wbt = np.ascontiguousarray(wbT.reshape(NHT, 128, K).transpose(1, 0, 2))
    wat = np.ascontiguousarray(
        Wa_w.reshape(NKT, 128).T).astype(np.float32)
    bias = np.ascontiguousarray(
        (Wb_b + Wc_b).reshape(NKT, 128).T).astype(np.float32)
    return wct, wbt, wat, bias


def kernel(decoder_state, encoder_hiddens, Wa_w, Wa_b, Wb_w, Wb_b, Wc_w, Wc_b,
           **run_kwargs):
    decoder_state = np.asarray(decoder_state, dtype=np.float32)
    encoder_hiddens = np.asarray(encoder_hiddens, dtype=np.float32)
    enc_bf16 = encoder_hiddens.astype(ml_dtypes.bfloat16)
    decT = np.ascontiguousarray(decoder_state.T).astype(ml_dtypes.bfloat16)
    wct, wbt, wat, bias = _prep_weights(
        np.asarray(Wa_w, dtype=np.float32),
        np.asarray(Wb_w, dtype=np.float32),
        np.asarray(Wb_b, dtype=np.float32),
        np.asarray(Wc_w, dtype=np.float32),
        np.asarray(Wc_b, dtype=np.float32))

    nc = _get_nc()
    in_maps = []
    for c in range(NCORES):
        dect = np.ascontiguousarray(
            decT[:, c * BLOC:(c + 1) * BLOC].reshape(NHT, 128, BLOC)
            .transpose(1, 0, 2))
        in_maps.append({
            "enc": np.ascontiguousarray(enc_bf16[c * BLOC:(c + 1) * BLOC]),
            "wct": wct,
            "wbt": wbt,
            "dect": dect,
            "wat": wat,
            "bias": bias,
        })
    res = bass_utils.run_bass_kernel_spmd(
        nc, in_maps, core_ids=list(range(NCORES)), **run_kwargs)
    out = np.concatenate([res.results[c]["y"] for c in range(NCORES)], axis=0)
    # Wa_b shifts every score equally; softmax is invariant to it.
    if run_kwargs:
        return out, res
    return out


# revision 22
# speedup vs baseline: 1.1692x; 1.1692x over previous
"""Bahdanau attention Trainium2 Bass kernel (v2).

Problem (fixed shapes):
  decoder_state [32, 1024] f32, encoder_hiddens [32, 2048, 1024] f32,
  Wa_w [1,1024], Wa_b [1], Wb_w [1024,1024], Wb_b [1024], Wc_w [1024,1024], Wc_b [1024]
  out: context [32, 1024] f32

Strategy: data-parallel over batch, 4 batches per core on 8 cores.

Host-side prep is layout-only: cast encoder_hiddens to bf16 (keeps DMA at
half volume) and pre-transpose/cast the small weight matrices into the
[h-partition, tile, free] layouts the PE wants. On-chip there are NO
enc/weight transposes: encoder blocks are loaded with the DMA XBAR
transpose (16-bit path) directly into [h, s] tiles.

Per 512-wide s-block: enc_proj k-tiles via bf16 matmuls (8 ht-accumulated
groups, moving dim 512), tanh(+dec_proj bias) on the Act engine into bf16
e-tiles, the Wa score reduction as a DVE multiply-accumulate over
k-tiles followed by a single ones-vector matmul (cross-partition sum),
exp without max subtraction (scores are O(+-5) for this input
distribution, fp32 exp is exact there; softmax shift-invariance makes
max subtraction optional), and the context partial on the DVE from the
same encT tiles. Score/context stages for block i are deferred into
block i+1's matmul stream so the PE never waits on them.
"""
import sys

if "/opt/trn_rl_repo" not in sys.path:
    sys.path.insert(0, "/opt/trn_rl_repo")

import numpy as np
import ml_dtypes

import concourse.bass as bass
import concourse.tile as tile
from concourse import bacc, mybir
from concourse import bass_utils
from concourse.masks import make_identity

F32 = mybir.dt.float32
F32R = mybir.dt.float32r
BF16 = mybir.dt.bfloat16
F16 = mybir.dt.float16

B, S, H, K = 32, 2048, 1024, 1024
NCORES = 8
BLOC = B // NCORES          # batches per core
SBLK = 512                  # s-block width
NBLK = S // SBLK            # 4
NHT = H // 128              # 8
NKT = K // 128              # 8
NIT = BLOC * NBLK           # 16 block iterations

# chunk schedule: (batch, s-offset, width, chunk-index-in-batch). The last
# block of the last batch is split into two 256-wide chunks to halve the
# serial score->context->combine chain at the kernel tail.
SCHED = []
for _b in range(BLOC):
    if _b < BLOC - 1:
        SCHED += [(_b, _blk * SBLK, SBLK, _blk) for _blk in range(NBLK)]
    else:
        SCHED += [(_b, 0, SBLK, 0), (_b, SBLK, SBLK, 1),
                  (_b, 2 * SBLK, SBLK, 2),
                  (_b, 3 * SBLK, 256, 3), (_b, 3 * SBLK + 256, 256, 4)]


def build_kernel():
    nc = bacc.Bacc("TRN2", target_bir_lowering=False)

    enc = nc.dram_tensor("enc", [BLOC, S, H], BF16, kind="ExternalInput")
    wct = nc.dram_tensor("wct", [128, NHT, K], BF16, kind="ExternalInput")
    wbt = nc.dram_tensor("wbt", [128, NHT, K], BF16, kind="ExternalInput")
    dect = nc.dram_tensor("dect", [128, NHT, BLOC], BF16, kind="ExternalInput")
    wat = nc.dram_tensor("wat", [128, NKT], F32, kind="ExternalInput")
    bias = nc.dram_tensor("bias", [128, NKT], F32, kind="ExternalInput")
    y = nc.dram_tensor("y", [BLOC, H], F32, kind="ExternalOutput")

    TT = mybir.ActivationFunctionType.Tanh
    EX = mybir.ActivationFunctionType.Exp
    ADD = mybir.AluOpType.add
    MULT = mybir.AluOpType.mult

    from contextlib import ExitStack
    with tile.TileContext(nc) as tc, ExitStack() as stack:
        consts = stack.enter_context(tc.tile_pool(name="consts", bufs=1))
        identf = consts.tile([128, 128], F32)
        make_identity(nc, identf)
        ones_col = consts.tile([128, 1], F16)
        nc.vector.memset(ones_col, 1.0)
        wcT = consts.tile([128, NHT, K], BF16)
        waT = consts.tile([128, NKT], F32)
        bseg = consts.tile([128, NKT], F32)
        decT = consts.tile([128, NHT, BLOC], BF16)
        bias_kb = consts.tile([128, NKT, BLOC], F32)

        enc_p = stack.enter_context(tc.tile_pool(name="encT", bufs=5))

        def load_enc(i):
            b, off, w, ci = SCHED[i]
            t = enc_p.tile([128, NHT, SBLK], BF16, tag="eT")
            nc.sync.dma_start_transpose(
                t[:, :, 0:w], enc[b, off:off + w, :])
            return t

        # ---------------- setup: dec_proj -> bias_kb ----------------
        # DMA order: wbT leads the sync ring (it gates the dec_proj -> bias
        # chain, which must beat the first tanh); the enc-block XBAR stream
        # follows it. wcT + smalls go on the scalar ring in parallel.
        setup_ctx = ExitStack()
        setup = setup_ctx.enter_context(tc.tile_pool(name="setup", bufs=1))
        sps = setup_ctx.enter_context(
            tc.tile_pool(name="setup_ps", bufs=1, space="PSUM"))
        if True:
            wbT = setup.tile([128, NHT, K], BF16, tag="wbT")
            nc.sync.dma_start(out=wbT[:, :, 0:512], in_=wbt[:, :, 0:512])
            nc.sync.dma_start(out=wbT[:, :, 512:K], in_=wbt[:, :, 512:K])
            nc.scalar.dma_start(out=decT, in_=dect[:, :, :])
            nc.scalar.dma_start(out=waT, in_=wat[:, :])
            nc.scalar.dma_start(out=bseg, in_=bias[:, :])
            nc.scalar.dma_start(out=wcT, in_=wct[:, :, :])
            encT = {0: load_enc(0), 1: load_enc(1)}

            dp_row = setup.tile([BLOC, K], F32, tag="dp_row")
            for half in range(2):
                psd = sps.tile([BLOC, 512], F32, tag=f"psd{half}")
                for ht in range(NHT):
                    nc.tensor.matmul(psd, decT[:, ht, :],
                                     wbT[:, ht, half * 512:(half + 1) * 512],
                                     start=(ht == 0), stop=(ht == NHT - 1))
                nc.vector.tensor_copy(dp_row[:, half * 512:(half + 1) * 512], psd)
            pst = sps.tile([128, NKT, BLOC], F32, tag="pst")
            for kt in range(NKT):
                nc.tensor.transpose(pst[:, kt, :],
                                    dp_row[:, kt * 128:(kt + 1) * 128],
                                    identf[0:BLOC, 0:BLOC])
            for kt in range(NKT):
                bs = bseg[:, kt:kt + 1]
                nc.vector.tensor_tensor(
                    out=bias_kb[:, kt, :], in0=pst[:, kt, :],
                    in1=bass.AP(tensor=bs.tensor, offset=bs.offset,
                                ap=[bs.ap[0], [0, BLOC]]),
                    op=ADD)
        setup_ctx.close()

        # ---------------- main loop pools ----------------
        e_p = stack.enter_context(tc.tile_pool(name="e", bufs=10))
        acc_p = stack.enter_context(tc.tile_pool(name="acc", bufs=2))
        wbt_p = stack.enter_context(tc.tile_pool(name="wbcast", bufs=2))
        scr_p = stack.enter_context(tc.tile_pool(name="scr", bufs=2))
        row_p = stack.enter_context(tc.tile_pool(name="rows", bufs=3))
        stat_p = stack.enter_context(tc.tile_pool(name="stats", bufs=6))
        ctx_p = stack.enter_context(tc.tile_pool(name="ctxT", bufs=8))
        ysb_p = stack.enter_context(tc.tile_pool(name="ysb", bufs=2))
        ps_e = stack.enter_context(tc.tile_pool(name="ps_e", bufs=6, space="PSUM"))
        ps_s = stack.enter_context(tc.tile_pool(name="ps_s", bufs=1, space="PSUM"))
        ps_y = stack.enter_context(tc.tile_pool(name="ps_y", bufs=1, space="PSUM"))

        def flush_scores(task):
            # chunk i's scores: cross-partition sum of acc via ones-matmul,
            # then exp (no max subtraction; see module docstring).
            i, acc, zrow, ctx_blks, w, ci = task
            pss = ps_s.tile([1, SBLK], F32, tag="pss")
            nc.tensor.matmul(pss[:, 0:w], ones_col, acc[:, 0:w],
                             start=True, stop=True)
            wrow = row_p.tile([1, SBLK], BF16, tag="wrow")
            nc.scalar.activation(wrow[:, 0:w], pss[:, 0:w], EX,
                                 accum_out=zrow[:, ci:ci + 1])
            return wrow

        def flush_context(task, wrow):
            # chunk i's context partial on DVE from the encT tiles.
            i, acc, zrow, ctx_blks, w, ci = task
            wb_t = wbt_p.tile([128, SBLK], BF16, tag="wb")
            nc.gpsimd.partition_broadcast(wb_t[:, 0:w], wrow[:, 0:w], 128)
            ctxT = ctx_p.tile([128, NHT], F32, tag="ct")
            for ht in range(NHT):
                scr = scr_p.tile([128, SBLK], BF16, tag="scr")
                nc.vector.scalar_tensor_tensor(
                    out=scr[:, 0:w], in0=encT[i][:, ht, 0:w], scalar=1.0,
                    in1=wb_t[:, 0:w],
                    op0=MULT, op1=MULT, accum_out=ctxT[:, ht:ht + 1])
            ctx_blks.append(ctxT)

        def flush_combine(task):
            b, zrow, ctx_blks = task
            z = stat_p.tile([1, 1], F32, tag="z")
            nc.vector.reduce_sum(z, zrow[:, 0:len(ctx_blks)],
                                 axis=mybir.AxisListType.X)
            rz = stat_p.tile([1, 1], F32, tag="rz")
            nc.vector.reciprocal(rz, z)
            rzB = stat_p.tile([128, 1], F32, tag="rzB")
            nc.gpsimd.partition_broadcast(rzB, rz, 128)
            s01 = stat_p.tile([128, NHT], F32, tag="s01")
            nc.vector.tensor_tensor(out=s01, in0=ctx_blks[0], in1=ctx_blks[1], op=ADD)
            s23 = stat_p.tile([128, NHT], F32, tag="s23")
            nc.vector.tensor_tensor(out=s23, in0=ctx_blks[2], in1=ctx_blks[3], op=ADD)
            stot = stat_p.tile([128, NHT], F32, tag="stot")
            nc.vector.tensor_tensor(out=stot, in0=s01, in1=s23, op=ADD)
            if len(ctx_blks) > 4:
                stot5 = stat_p.tile([128, NHT], F32, tag="stot5")
                nc.vector.tensor_tensor(out=stot5, in0=stot, in1=ctx_blks[4], op=ADD)
                stot = stot5
            ys = stat_p.tile([128, NHT], F32, tag="ys")
            nc.vector.tensor_scalar_mul(ys, stot, rzB)
            psy = ps_y.tile([NHT, 128], F32, tag="psy")
            nc.tensor.transpose(psy, ys, identf)
            yrow = ysb_p.tile([NHT, 128], F32, tag="yrow")
            nc.vector.tensor_copy(yrow, psy)
            nc.scalar.dma_start(
                out=y[b:b + 1, :].rearrange("o (ht hp) -> (o ht) hp", hp=128),
                in_=yrow)

        pending = None        # task awaiting scores+context
        pending_comb = None   # (b, zrow, ctx_blks) awaiting final combine
        zrow = None
        ctx_blks = None
        for i, (b, off, w, ci) in enumerate(SCHED):
            if ci == 0:
                zrow = stat_p.tile([1, NBLK + 1], F32, tag="zrow")
                ctx_blks = []
            if i + 2 < len(SCHED):
                encT[i + 2] = load_enc(i + 2)

            acc = acc_p.tile([128, SBLK], F16, tag="acc")
            for kt in range(NKT):
                pse = ps_e.tile([128, SBLK], F32, tag="pe")
                for ht in range(NHT):
                    nc.tensor.matmul(pse[:, 0:w],
                                     wcT[:, ht, kt * 128:(kt + 1) * 128],
                                     encT[i][:, ht, 0:w],
                                     start=(ht == 0), stop=(ht == NHT - 1))
                et = e_p.tile([128, SBLK], BF16, tag="et")
                nc.scalar.activation(et[:, 0:w], pse[:, 0:w], TT,
                                     bias=bias_kb[:, kt, b:b + 1])
                if kt == 0:
                    nc.vector.tensor_scalar_mul(acc[:, 0:w], et[:, 0:w],
                                                waT[:, 0:1])
                else:
                    nc.vector.scalar_tensor_tensor(
                        out=acc[:, 0:w], in0=et[:, 0:w],
                        scalar=waT[:, kt:kt + 1], in1=acc[:, 0:w],
                        op0=MULT, op1=ADD)

                if kt == 2 and pending is not None:
                    wrow = flush_scores(pending)
                if kt == 4 and pending is not None:
                    flush_context(pending, wrow)
                    del encT[pending[0]]
                    pending = None
                if kt == 6 and pending_comb is not None:
                    flush_combine(pending_comb)
                    pending_comb = None

            pending = (i, acc, zrow, ctx_blks, w, ci)
            if i + 1 == len(SCHED) or SCHED[i + 1][3] == 0:
                pending_comb = (b, zrow, ctx_blks)

        wrow = flush_scores(pending)
        flush_context(pending, wrow)
        flush_combine(pending_comb)

    nc.compile()
    return nc


_NC_CACHE = None


def _get_nc():
    global _NC_CACHE
    if _NC_CACHE is None:
        _NC_CACHE = build_kernel()
    return _NC_CACHE


def _prep_weights(Wa_w, Wb_w, Wb_b, Wc_w, Wc_b):
    # [h, k] transposed weights, rows regrouped to [128, NHT, K] with
    # h = ht*128 + p (matches the XBAR DMA-transpose layout of enc tiles).
    wcT = np.ascontiguousarray(Wc_w.T).astype(ml_dtypes.bfloat16)
    wbT = np.ascontiguousarray(Wb_w.T).astype(ml_dtypes.bfloat16)
    wct = np.ascontiguousarray(wcT.reshape(NHT, 128, K).transpose(1, 0, 2))
    wbt = np.ascontiguousarray(wbT.reshape(NHT, 128, K).transpose(1, 0, 2))
    wat = np.ascontiguousarray(
        Wa_w.reshape(NKT, 128).T).astype(np.float32)
    bias = np.ascontiguousarray(
        (Wb_b + Wc_b).reshape(NKT, 128).T).astype(np.float32)
    return wct, wbt, wat, bias


def kernel(decoder_state, encoder_hiddens, Wa_w, Wa_b, Wb_w, Wb_b, Wc_w, Wc_b,
           **run_kwargs):
    decoder_state = np.asarray(decoder_state, dtype=np.float32)
    encoder_hiddens = np.asarray(encoder_hiddens, dtype=np.float32)
    enc_bf16 = encoder_hiddens.astype(ml_dtypes.bfloat16)
    decT = np.ascontiguousarray(decoder_state.T).astype(ml_dtypes.bfloat16)
    wct, wbt, wat, bias = _prep_weights(
        np.asarray(Wa_w, dtype=np.float32),
        np.asarray(Wb_w, dtype=np.float32),
        np.asarray(Wb_b, dtype=np.float32),
        np.asarray(Wc_w, dtype=np.float32),
        np.asarray(Wc_b, dtype=np.float32))

    nc = _get_nc()
    in_maps = []
    for c in range(NCORES):
        dect = np.ascontiguousarray(
            decT[:, c * BLOC:(c + 1) * BLOC].reshape(NHT, 128, BLOC)
            .transpose(1, 0, 2))
        in_maps.append({
            "enc": np.ascontiguousarray(enc_bf16[c * BLOC:(c + 1) * BLOC]),
            "wct": wct,
            "wbt": wbt,
            "dect": dect,
            "wat": wat,
            "bias": bias,
        })
    res = bass_utils.run_bass_kernel_spmd(
        nc, in_maps, core_ids=list(range(NCORES)), **run_kwargs)
    out = np.concatenate([res.results[c]["y"] for c in range(NCORES)], axis=0)
    # Wa_b shifts every score equally; softmax is invariant to it.
    if run_kwargs:
        return out, res
    return out
